# revision 16
# baseline (speedup 1.0000x reference)
"""Trainium2 Bass kernel for nn_CubicModelLarge (3-layer cubic-feature MLP).

Tensor-parallel over the cubic multiplier index i (64 values, 8 per core).
The cubic expansion is never materialized.  Per layer:

  y[b,o] = W_lin@x + b + sum_t W_sq[o,t] xsq[b,t] + sum_i x[b,i] sum_t W_cu[o,i,t] xsq[b,t]

Per core c (i in I_c = [8c, 8c+8)):

  H[b,(il,o)] = sum_J F[J,b] * Wcub[J,(il,o)]     (fp16 GEMM, J = 17x128 rows)
  y_c[b,o]    = lin[b,o] + b + sum_il x[b, i(il)] * H[b,(il,o)]
  y = AllReduce_c(y_c)

F chunks (128 rows each, per half-batch of 512):
  k=0..15 : [rot(2k+1); rot(2k+2)] products  x_a*x_{(a+d)%64}, d = 2k+1 + p//64
  k=16    : [x rows (carrying the symmetrized W_sq fold); squares x_a^2]
Rotated copies are built with PE selection matmuls -> ACT copy to fp16 SBUF ->
DVE 2x-mode products.  x^T itself (X2 = [x^T; x^T]) comes from a single
xbar transpose-DMA of the column-duplicated AllReduce payload (512, 128) --
no PE transposes anywhere.  The GEMM runs chunk-outer across 4 batch-chunk
PSUM banks so PE stays dense; per-sample combine is DVE scalar_tensor_tensor
with x slices taken batch-major (no selection matmuls for xmac).
Final-layer partials are summed on the host in fp32.
"""

import numpy as np

F16NP = np.float16

D = 64
B = 1024
NCORES = 8
I_PER = D // NCORES          # 8
OUTS = (64, 64, 10)
NK = 16                      # rotation-pair chunks
HB = 512                     # half-batch
NBH = HB // 128              # 4 batch chunks per half
PIPE = 2                     # rep-matmul software pipeline depth

_CACHE = {}


# ---------------------------------------------------------------- host prep --

def _maps():
    iu, ju = np.triu_indices(D)
    tmap = np.zeros((D, D), np.int64)
    tmap[iu, ju] = np.arange(len(iu))
    tmap[ju, iu] = tmap[iu, ju]
    p = np.arange(128)
    rows_t = np.zeros((NK, 128), np.int64)
    for k in range(NK):
        d = 2 * k + 1 + p // 64
        a = p % 64
        rows_t[k] = tmap[a, (a + d) % D]
    diag_t = tmap[np.arange(D), np.arange(D)]
    return tmap, rows_t, diag_t


def _prep_layer(W, b, out):
    """-> wcub [NCORES](17*128, I_PER*out) fp16, wlin [NCORES](64, out) fp16"""
    _, rows_t, diag_t = _maps()
    W_lin = W[:, :D]
    W_sq = W[:, D:D + 2080]
    W_cu = W[:, D + 2080:].reshape(out, D, 2080)

    iu, ju = np.triu_indices(D)
    w2 = np.zeros((out, D, D), np.float32)
    half = np.where(iu == ju, 1.0, 0.5).astype(np.float32)
    w2[:, iu, ju] = W_sq * half
    w2[:, ju, iu] = W_sq * half

    rt = rows_t.reshape(-1)
    # gap-32 rows (upper half of chunk 15) are double-counted -> halve
    scale = np.ones(NK * 128, np.float32)
    scale[15 * 128 + 64:16 * 128] = 0.5

    wcubs, wlins = [], []
    for core in range(NCORES):
        I = np.arange(core * I_PER, (core + 1) * I_PER)
        M = I_PER * out
        wcub = np.zeros((17 * 128, M), np.float32)
        blk = W_cu[:, I, :][:, :, rt] * scale[None, None, :]
        wcub[:NK * 128] = blk.transpose(2, 1, 0).reshape(NK * 128, M)
        w2blk = w2[:, I, :]                                 # (out, I_PER, 64)
        wcub[NK * 128:NK * 128 + D] = w2blk.transpose(2, 1, 0).reshape(D, M)
        dblk = W_cu[:, I, :][:, :, diag_t]
        wcub[NK * 128 + D:] = dblk.transpose(2, 1, 0).reshape(D, M)
        wcubs.append(np.ascontiguousarray(wcub.astype(F16NP)))

        wl = np.zeros((D, out), np.float32)
        if core == 0:
            wl[:] = W_lin.T
        wlins.append(np.ascontiguousarray(wl.astype(F16NP)))
    return wcubs, wlins


def _sel_consts():
    """PE selection matrices (64, NK*128): slot k -> [rot(2k+1); rot(2k+2)]."""
    sel = np.zeros((D, NK * 128), np.float32)
    for k in range(NK):
        for p in range(128):
            d = 2 * k + 1 + p // 64
            a = p % 64
            sel[(a + d) % D, k * 128 + p] = 1.0
    return sel


# ------------------------------------------------------------------ builder --

def _build_module():
    import concourse.bacc as bacc
    import concourse.mybir as mybir
    import concourse.tile as tile

    F32 = mybir.dt.float32
    F16 = mybir.dt.float16
    MULT = mybir.AluOpType.mult
    ADD = mybir.AluOpType.add

    nc = bacc.Bacc("TRN2", target_bir_lowering=False, num_devices=NCORES, debug=False)

    xdup_in = nc.dram_tensor("xdup", [B, 128], F16, kind="ExternalInput")
    colsel_in = nc.dram_tensor("colsel", [D, I_PER], F16, kind="ExternalInput")
    xmac0_in = nc.dram_tensor("xmac0", [128, B // 128, I_PER], F16, kind="ExternalInput")
    wcub_in = [
        nc.dram_tensor(f"wcub{li}", [17 * 128, I_PER * OUTS[li]], F16, kind="ExternalInput")
        for li in range(3)
    ]
    wlin_in = [
        nc.dram_tensor(f"wlin{li}", [D, OUTS[li]], F16, kind="ExternalInput")
        for li in range(3)
    ]
    bfull_in = [
        nc.dram_tensor(f"bfull{li}", [128, NBH, OUTS[li]], F16, kind="ExternalInput")
        for li in range(3)
    ]
    out_ext = nc.dram_tensor("out", [B, OUTS[2]], F32, kind="ExternalOutput")

    sel_c = nc.inline_tensor(_sel_consts().astype(np.float16), name="selc")
    ident16_c = nc.inline_tensor(np.eye(128, dtype=np.float16), name="ident16c")

    with tile.TileContext(nc) as tc:
        with (
            tc.tile_pool(name="spool", bufs=1) as spool,
            tc.tile_pool(name="wpool", bufs=3) as wpool,
            tc.tile_pool(name="xpool", bufs=2) as xpool,
            tc.tile_pool(name="fpool", bufs=4) as fpool,
            tc.tile_pool(name="qpool", bufs=3) as qpool,
            tc.tile_pool(name="hpool", bufs=2) as hpool,
            tc.tile_pool(name="ypool", bufs=2) as ypool,
            tc.tile_pool(name="ps_rep", bufs=2, space="PSUM") as ps_rep,
            tc.tile_pool(name="ps_h", bufs=1, space="PSUM") as ps_h,
            tc.tile_pool(name="ps_small", bufs=2, space="PSUM") as ps_small,
            tc.tile_pool(name="dpool", bufs=2, space="DRAM") as dpool,
        ):
            # layer-0 activations first on the sync queue so L0 starts ASAP
            x2_t, xmac_t = [None, None], [None, None]
            for h in range(2):
                x2 = xpool.tile([128, HB], F16, tag=f"x2h{h}")
                nc.sync.dma_start(x2[:], xdup_in.ap()[h * HB:(h + 1) * HB, :],
                                  transpose=True)
                xmac = xpool.tile([128, NBH, I_PER], F16, tag=f"xmh{h}")
                nc.sync.dma_start(
                    xmac[:], xmac0_in.ap()[:, h * NBH:(h + 1) * NBH, :])
                x2_t[h], xmac_t[h] = x2, xmac

            sel_sb = spool.tile([D, NK * 128], F16, tag="sel")
            nc.scalar.dma_start(sel_sb[:], sel_c.ap())
            ident16_sb = spool.tile([128, 128], F16, tag="ident16")
            nc.scalar.dma_start(ident16_sb[:], ident16_c.ap())
            bfull_sb = []
            for li in range(3):
                bt = spool.tile([128, NBH, OUTS[li]], F16, tag=f"bfull{li}")
                nc.scalar.dma_start(bt[:], bfull_in[li].ap())
                bfull_sb.append(bt)

            colsel_sb = spool.tile([D, I_PER], F16, tag="colsel")
            nc.scalar.dma_start(colsel_sb[:], colsel_in.ap())

            # weights: two big strided DMAs per layer (keep the ACT queue clear)
            weights = []
            for li in range(3):
                M = I_PER * OUTS[li]
                wcub_sb = wpool.tile([128, 17, M], F16, tag="wcub")
                src = wcub_in[li].ap().rearrange("(k p) m -> p k m", p=128)
                nc.scalar.dma_start(wcub_sb[:, 0:2, :], src[:, 0:2, :])
                nc.scalar.dma_start(wcub_sb[:, 2:17, :], src[:, 2:17, :])
                wlin_sb = wpool.tile([D, OUTS[li]], F16, tag="wlin")
                nc.scalar.dma_start(wlin_sb[:], wlin_in[li].ap())
                weights.append((wcub_sb, wlin_sb))

            for li in range(3):
                out_l = OUTS[li]
                M = I_PER * out_l
                last = li == 2
                wcub_sb, wlin_sb = weights[li]

                for h in range(2):
                    x2 = x2_t[h]
                    # lin and xmac share one PSUM bank: (128, NBH, out_l + I_PER)
                    lin_ps = ps_small.tile([128, NBH, out_l + I_PER], F32, tag="lin")
                    if li == 0:
                        xmac = xmac_t[h]
                    else:
                        # extract this core's x_i columns (batch-major) on PE
                        for bc in range(NBH):
                            bs = slice(bc * 128, (bc + 1) * 128)
                            nc.tensor.matmul(lin_ps[:, bc, out_l:], x2[0:D, bs],
                                             colsel_sb[:], start=True, stop=True)
                        xmac = xpool.tile([128, NBH, I_PER], F16, tag=f"xmh{h}")
                        nc.scalar.copy(xmac[:], lin_ps[:, :, out_l:])

                    # rep matmul -> ACT copy -> DVE product, software-pipelined
                    # against the chunk-outer GEMM
                    if not last:
                        hps = [ps_h.tile([128, M], F32, tag=f"h{bc}", name=f"hps{bc}")
                               for bc in range(NBH)]
                    else:
                        h2_ps = ps_h.tile([M, HB], F32, tag="h0")

                    fks = [None] * 17

                    def make_fk(k):
                        if k < NK:
                            rep = ps_rep.tile([128, HB], F32, tag="rep")
                            nc.tensor.matmul(
                                rep[:], sel_sb[:, k * 128:(k + 1) * 128],
                                x2[0:D, :], start=True, stop=True,
                            )
                            rk = qpool.tile([128, HB], F16, tag="rk")
                            nc.scalar.copy(rk[:], rep[:])
                            fk = fpool.tile([128, HB], F16, tag="f")
                            nc.vector.tensor_mul(fk[:], x2[:], rk[:])
                        else:
                            fk = fpool.tile([128, HB], F16, tag="f")
                            nc.scalar.copy(fk[0:D, :], x2[0:D, :])
                            nc.vector.tensor_mul(fk[D:128, :], x2[D:128, :], x2[D:128, :])
                        fks[k] = fk

                    def consume_fk(k):
                        fk = fks[k]
                        first, last_k = k == 0, k == 16
                        if not last:
                            for bc in range(NBH):
                                bs = slice(bc * 128, (bc + 1) * 128)
                                nc.tensor.matmul(
                                    hps[bc][:], fk[:, bs], wcub_sb[:, k, :],
                                    start=first, stop=last_k,
                                )
                        else:
                            nc.tensor.matmul(
                                h2_ps[:], wcub_sb[:, k, :], fk[:],
                                start=first, stop=last_k,
                            )

                    for k in range(17 + PIPE):
                        if k < 17:
                            make_fk(k)
                        if k >= PIPE:
                            consume_fk(k - PIPE)

                    # linear part + bias
                    for bc in range(NBH):
                        bs = slice(bc * 128, (bc + 1) * 128)
                        nc.tensor.matmul(lin_ps[:, bc, 0:out_l], x2[0:D, bs],
                                         wlin_sb[:], start=True, stop=True)

                    y_sb = ypool.tile([128, NBH, out_l], F32 if last else F16,
                                      tag=f"y{'2' if last else ''}")
                    nc.vector.tensor_add(y_sb[:], lin_ps[:, :, 0:out_l], bfull_sb[li][:])

                    if not last:
                        hsb = hpool.tile([128, NBH, M], F16, tag="hs")
                        for bc in range(NBH):
                            nc.scalar.copy(hsb[:, bc, :], hps[bc][:])
                        for bc in range(NBH):
                            for il in range(I_PER):
                                nc.vector.scalar_tensor_tensor(
                                    y_sb[:, bc, :],
                                    hsb[:, bc, il * out_l:(il + 1) * out_l],
                                    xmac[:, bc, il:il + 1],
                                    y_sb[:, bc, :],
                                    op0=MULT, op1=ADD,
                                )

                        # AllReduce (512, 64), then local dup + transpose-DMA back
                        bounce = dpool.tile([HB, D], F16, tag=f"bounce{h}")
                        red = dpool.tile([HB, D], F16, tag=f"red{h}")
                        red2 = dpool.tile([HB, 128], F16, tag=f"red2{h}")
                        nc.sync.dma_start(
                            bounce[:].rearrange("(bc p) o -> p bc o", p=128),
                            y_sb[:],
                        )
                        nc.gpsimd.collective_compute(
                            "AllReduce",
                            ADD,
                            replica_groups=[list(range(NCORES))],
                            ins=[bounce.opt()],
                            outs=[red.opt()],
                        )
                        for cw in range(2):
                            nc.sync.dma_start(
                                red2[:, cw * D:(cw + 1) * D], red[:])
                        x2n = xpool.tile([128, HB], F16, tag=f"x2h{h}")
                        nc.sync.dma_start(x2n[:], red2[:], transpose=True)
                        x2_t[h] = x2n
                    else:
                        h2_sb = hpool.tile([M, HB], F16, tag="hs")
                        nc.scalar.copy(h2_sb[:], h2_ps[:])
                        for bc in range(NBH):
                            bs = slice(bc * 128, (bc + 1) * 128)
                            t_ps = ps_rep.tile([128, M], F16, tag="rep")
                            nc.tensor.transpose(t_ps[:], h2_sb[:, bs],
                                                ident16_sb[0:M, 0:M])
                            h2c = qpool.tile([128, M], F16, tag="rk")
                            nc.scalar.copy(h2c[:], t_ps[:])
                            for il in range(I_PER):
                                nc.vector.scalar_tensor_tensor(
                                    y_sb[:, bc, :],
                                    h2c[:, il * out_l:(il + 1) * out_l],
                                    xmac[:, bc, il:il + 1],
                                    y_sb[:, bc, :],
                                    op0=MULT, op1=ADD,
                                )
                        nc.sync.dma_start(
                            out_ext.ap()[h * HB:(h + 1) * HB, :]
                            .rearrange("(bc p) o -> p bc o", p=128),
                            y_sb[:],
                        )

    nc.compile()
    return nc


# ------------------------------------------------------------------- runner --

def build_in_maps(x, W0, b0, W1, b1, W2, b2):
    x = np.asarray(x, np.float32)
    Ws = [np.asarray(W, np.float32) for W in (W0, W1, W2)]
    bs = [np.asarray(b_, np.float32) for b_ in (b0, b1, b2)]

    x16 = x.astype(F16NP)
    xdup = np.concatenate([x16, x16], axis=1)          # (B, 128)

    wcubs, wlins = {}, {}
    for li in range(3):
        wcubs[li], wlins[li] = _prep_layer(Ws[li], bs[li], OUTS[li])

    in_maps = []
    for core in range(NCORES):
        I = np.arange(core * I_PER, (core + 1) * I_PER)
        colsel = np.zeros((D, I_PER), np.float32)
        colsel[I, np.arange(I_PER)] = 1.0
        xmac0 = np.ascontiguousarray(
            x16[:, I].reshape(B // 128, 128, I_PER).transpose(1, 0, 2))
        m = {"xdup": xdup, "colsel": colsel.astype(F16NP), "xmac0": xmac0}
        for li in range(3):
            m[f"wcub{li}"] = wcubs[li][core]
            m[f"wlin{li}"] = wlins[li][core]
            bf = np.zeros((128, NBH, OUTS[li]), np.float32)
            if core == 0:
                bf[:] = bs[li][None, None, :]
            m[f"bfull{li}"] = bf.astype(F16NP)
        in_maps.append(m)
    return in_maps


def kernel(x, W0, b0, W1, b1, W2, b2):
    from concourse.bass_utils import run_bass_kernel_spmd

    if "nc" not in _CACHE:
        _CACHE["nc"] = _build_module()
    nc = _CACHE["nc"]

    in_maps = build_in_maps(x, W0, b0, W1, b1, W2, b2)
    res = run_bass_kernel_spmd(nc, in_maps, core_ids=list(range(NCORES)))
    out = np.zeros((B, OUTS[2]), np.float32)
    for core in range(NCORES):
        out += res.results[core]["out"]
    return out


# revision 18
# speedup vs baseline: 1.3211x; 1.3211x over previous
"""Trainium2 Bass kernel for nn_CubicModelLarge (3-layer cubic-feature MLP).

Tensor-parallel over the cubic multiplier index i (64 values, 8 per core).
The cubic expansion is never materialized.  Per layer:

  y[b,o] = W_lin@x + b + sum_t W_sq[o,t] xsq[b,t] + sum_i x[b,i] sum_t W_cu[o,i,t] xsq[b,t]

Per core c (i in I_c = [8c, 8c+8)):

  H[b,(il,o)] = sum_J F[J,b] * Wcub[J,(il,o)]     (fp16 GEMM, J = 17x128 rows)
  y_c[b,o]    = lin[b,o] + b + sum_il x[b, i(il)] * H[b,(il,o)]
  y = AllReduce_c(y_c)

F chunks (128 rows each, per half-batch of 512):
  k=0..15 : [rot(2k+1); rot(2k+2)] products  x_a*x_{(a+d)%64}, d = 2k+1 + p//64
  k=16    : [x rows (carrying the symmetrized W_sq fold); squares x_a^2]
Rotated copies are built with PE selection matmuls -> ACT copy to fp16 SBUF ->
DVE 2x-mode products.  x^T itself (X2 = [x^T; x^T]) comes from a single
xbar transpose-DMA of the column-duplicated AllReduce payload (512, 128) --
no PE transposes anywhere.  The GEMM runs chunk-outer across 4 batch-chunk
PSUM banks so PE stays dense; per-sample combine is DVE scalar_tensor_tensor
with x slices taken batch-major (no selection matmuls for xmac).
Final-layer partials are summed on the host in fp32.
"""

import numpy as np

F16NP = np.float16

D = 64
B = 1024
NCORES = 8
I_PER = D // NCORES          # 8
OUTS = (64, 64, 10)
NK = 16                      # rotation-pair chunks
HB = 512                     # half-batch
NBH = HB // 128              # 4 batch chunks per half
PIPE = 2                     # rep-matmul software pipeline depth

_CACHE = {}


# ---------------------------------------------------------------- host prep --

def _maps():
    iu, ju = np.triu_indices(D)
    tmap = np.zeros((D, D), np.int64)
    tmap[iu, ju] = np.arange(len(iu))
    tmap[ju, iu] = tmap[iu, ju]
    p = np.arange(128)
    rows_t = np.zeros((NK, 128), np.int64)
    for k in range(NK):
        d = 2 * k + 1 + p // 64
        a = p % 64
        rows_t[k] = tmap[a, (a + d) % D]
    diag_t = tmap[np.arange(D), np.arange(D)]
    return tmap, rows_t, diag_t


def _prep_layer(W, b, out):
    """-> wcub [NCORES](17*128, I_PER*out) fp16, wlin [NCORES](64, out) fp16"""
    _, rows_t, diag_t = _maps()
    W_lin = W[:, :D]
    W_sq = W[:, D:D + 2080]
    W_cu = W[:, D + 2080:].reshape(out, D, 2080)

    iu, ju = np.triu_indices(D)
    w2 = np.zeros((out, D, D), np.float32)
    half = np.where(iu == ju, 1.0, 0.5).astype(np.float32)
    w2[:, iu, ju] = W_sq * half
    w2[:, ju, iu] = W_sq * half

    rt = rows_t.reshape(-1)
    # gap-32 rows (upper half of chunk 15) are double-counted -> halve
    scale = np.ones(NK * 128, np.float32)
    scale[15 * 128 + 64:16 * 128] = 0.5

    wcubs, wlins = [], []
    for core in range(NCORES):
        I = np.arange(core * I_PER, (core + 1) * I_PER)
        M = I_PER * out
        # column order m = o * I_PER + il so the combine can reduce over
        # il as the innermost (stride-1) axis
        wcub = np.zeros((17 * 128, M), np.float32)
        blk = W_cu[:, I, :][:, :, rt] * scale[None, None, :]
        wcub[:NK * 128] = blk.transpose(2, 0, 1).reshape(NK * 128, M)
        w2blk = w2[:, I, :]                                 # (out, I_PER, 64)
        wcub[NK * 128:NK * 128 + D] = w2blk.transpose(2, 0, 1).reshape(D, M)
        dblk = W_cu[:, I, :][:, :, diag_t]
        wcub[NK * 128 + D:] = dblk.transpose(2, 0, 1).reshape(D, M)
        wcubs.append(np.ascontiguousarray(wcub.astype(F16NP)))

        wl = np.zeros((D, out), np.float32)
        if core == 0:
            wl[:] = W_lin.T
        wlins.append(np.ascontiguousarray(wl.astype(F16NP)))
    return wcubs, wlins


def _sel_consts():
    """PE selection matrices (64, NK*128): slot k -> [rot(2k+1); rot(2k+2)]."""
    sel = np.zeros((D, NK * 128), np.float32)
    for k in range(NK):
        for p in range(128):
            d = 2 * k + 1 + p // 64
            a = p % 64
            sel[(a + d) % D, k * 128 + p] = 1.0
    return sel


# ------------------------------------------------------------------ builder --

def _build_module():
    import concourse.bacc as bacc
    import concourse.mybir as mybir
    import concourse.tile as tile

    F32 = mybir.dt.float32
    F16 = mybir.dt.float16
    MULT = mybir.AluOpType.mult
    ADD = mybir.AluOpType.add

    nc = bacc.Bacc("TRN2", target_bir_lowering=False, num_devices=NCORES, debug=False)

    xdup_in = nc.dram_tensor("xdup", [B, 128], F16, kind="ExternalInput")
    colsel_in = nc.dram_tensor("colsel", [D, I_PER], F16, kind="ExternalInput")
    xmac0_in = nc.dram_tensor("xmac0", [128, B // 128, I_PER], F16, kind="ExternalInput")
    wcub_in = [
        nc.dram_tensor(f"wcub{li}", [17 * 128, I_PER * OUTS[li]], F16, kind="ExternalInput")
        for li in range(3)
    ]
    wlin_in = [
        nc.dram_tensor(f"wlin{li}", [D, OUTS[li]], F16, kind="ExternalInput")
        for li in range(3)
    ]
    bfull_in = [
        nc.dram_tensor(f"bfull{li}", [128, NBH, OUTS[li]], F16, kind="ExternalInput")
        for li in range(3)
    ]
    out_ext = nc.dram_tensor("out", [B, OUTS[2]], F32, kind="ExternalOutput")

    sel_c = nc.inline_tensor(_sel_consts().astype(np.float16), name="selc")
    ident16_c = nc.inline_tensor(np.eye(128, dtype=np.float16), name="ident16c")

    with tile.TileContext(nc) as tc:
        with (
            tc.tile_pool(name="spool", bufs=1) as spool,
            tc.tile_pool(name="wpool", bufs=3) as wpool,
            tc.tile_pool(name="xpool", bufs=2) as xpool,
            tc.tile_pool(name="fpool", bufs=4) as fpool,
            tc.tile_pool(name="qpool", bufs=3) as qpool,
            tc.tile_pool(name="hpool", bufs=2) as hpool,
            tc.tile_pool(name="ypool", bufs=2) as ypool,
            tc.tile_pool(name="ps_rep", bufs=2, space="PSUM") as ps_rep,
            tc.tile_pool(name="ps_h", bufs=1, space="PSUM") as ps_h,
            tc.tile_pool(name="ps_small", bufs=2, space="PSUM") as ps_small,
            tc.tile_pool(name="dpool", bufs=2, space="DRAM") as dpool,
        ):
            # layer-0 activations first on the sync queue so L0 starts ASAP
            x2_t, xmac_t = [None, None], [None, None]
            xs_t = [None, None]
            for h in range(2):
                x2 = xpool.tile([128, HB], F16, tag=f"x2h{h}")
                nc.sync.dma_start(x2[:], xdup_in.ap()[h * HB:(h + 1) * HB, :],
                                  transpose=True)
                xmac = xpool.tile([128, NBH, I_PER], F16, tag=f"xmh{h}")
                nc.sync.dma_start(
                    xmac[:], xmac0_in.ap()[:, h * NBH:(h + 1) * NBH, :])
                x2_t[h], xmac_t[h] = x2, xmac

            sel_sb = spool.tile([D, NK * 128], F16, tag="sel")
            nc.scalar.dma_start(sel_sb[:], sel_c.ap())
            ident16_sb = spool.tile([128, 128], F16, tag="ident16")
            nc.scalar.dma_start(ident16_sb[:], ident16_c.ap())
            bfull_sb = []
            for li in range(3):
                bt = spool.tile([128, NBH, OUTS[li]], F16, tag=f"bfull{li}")
                nc.scalar.dma_start(bt[:], bfull_in[li].ap())
                bfull_sb.append(bt)

            colsel_sb = spool.tile([D, I_PER], F16, tag="colsel")
            nc.scalar.dma_start(colsel_sb[:], colsel_in.ap())

            # weights: two big strided DMAs per layer (keep the ACT queue clear)
            weights = []
            for li in range(3):
                M = I_PER * OUTS[li]
                wcub_sb = wpool.tile([128, 17, M], F16, tag="wcub")
                src = wcub_in[li].ap().rearrange("(k p) m -> p k m", p=128)
                nc.scalar.dma_start(wcub_sb[:, 0:2, :], src[:, 0:2, :])
                nc.scalar.dma_start(wcub_sb[:, 2:17, :], src[:, 2:17, :])
                wlin_sb = wpool.tile([D, OUTS[li]], F16, tag="wlin")
                nc.scalar.dma_start(wlin_sb[:], wlin_in[li].ap())
                weights.append((wcub_sb, wlin_sb))

            for li in range(3):
                out_l = OUTS[li]
                M = I_PER * out_l
                last = li == 2
                wcub_sb, wlin_sb = weights[li]

                for h in range(2):
                    if li == 0:
                        x2 = x2_t[h]
                        xmac = xmac_t[h]
                        lin_ps = ps_small.tile(
                            [128, NBH, out_l + I_PER], F32, tag="lin")
                    else:
                        # rebuild x2 = [x^T; x^T] from the batch-major AR
                        # result with PE transposes (in the consumer body so
                        # the PE queue never head-of-line blocks on the AR)
                        xs = xs_t[h]
                        x2 = xpool.tile([128, HB], F16, tag=f"x2h{h}")
                        for bc in range(NBH):
                            bs = slice(bc * 128, (bc + 1) * 128)
                            tp = ps_rep.tile([D, 128], F16, tag="rep")
                            nc.tensor.transpose(tp[:], xs[:, bc, :], ident16_sb[:])
                            nc.scalar.copy(x2[0:D, bs], tp[:])
                        nc.scalar.copy(x2[D:128, :], x2[0:D, :])

                        lin_ps = ps_small.tile(
                            [128, NBH, out_l + I_PER], F32, tag="lin")
                        for bc in range(NBH):
                            bs = slice(bc * 128, (bc + 1) * 128)
                            nc.tensor.matmul(lin_ps[:, bc, out_l:], x2[0:D, bs],
                                             colsel_sb[:], start=True, stop=True)
                        xmac = xpool.tile([128, NBH, I_PER], F16, tag=f"xmh{h}")
                        nc.scalar.copy(xmac[:], lin_ps[:, :, out_l:])

                    # rep matmul -> product, software-pipelined against the
                    # chunk-outer GEMM; products alternate ACT-copy+2x-DVE and
                    # direct-PSUM 1x-DVE to balance the two engines
                    if not last:
                        hps = [ps_h.tile([128, M], F32, tag=f"h{bc}", name=f"hps{bc}")
                               for bc in range(NBH)]
                    else:
                        h2_ps = ps_h.tile([M, HB], F32, tag="h0")

                    fks = [None] * 17

                    def make_fk(k):
                        if k < NK:
                            rep = ps_rep.tile([128, HB], F32, tag="rep")
                            nc.tensor.matmul(
                                rep[:], sel_sb[:, k * 128:(k + 1) * 128],
                                x2[0:D, :], start=True, stop=True,
                            )
                            fk = fpool.tile([128, HB], F16, tag="f")
                            if k % 2 == 0:
                                rk = qpool.tile([128, HB], F16, tag="rk")
                                nc.scalar.copy(rk[:], rep[:])
                                nc.vector.tensor_mul(fk[:], x2[:], rk[:])
                            else:
                                nc.vector.tensor_mul(fk[:], x2[:], rep[:])
                        else:
                            fk = fpool.tile([128, HB], F16, tag="f")
                            nc.scalar.copy(fk[0:D, :], x2[0:D, :])
                            nc.vector.tensor_mul(fk[D:128, :], x2[D:128, :], x2[D:128, :])
                        fks[k] = fk

                    def consume_fk(k):
                        fk = fks[k]
                        first, last_k = k == 0, k == 16
                        if not last:
                            for bc in range(NBH):
                                bs = slice(bc * 128, (bc + 1) * 128)
                                nc.tensor.matmul(
                                    hps[bc][:], fk[:, bs], wcub_sb[:, k, :],
                                    start=first, stop=last_k,
                                )
                        else:
                            nc.tensor.matmul(
                                h2_ps[:], wcub_sb[:, k, :], fk[:],
                                start=first, stop=last_k,
                            )

                    for k in range(17 + PIPE):
                        if k < 17:
                            make_fk(k)
                        if k >= PIPE:
                            consume_fk(k - PIPE)

                    # linear part + bias -> y_base
                    for bc in range(NBH):
                        bs = slice(bc * 128, (bc + 1) * 128)
                        nc.tensor.matmul(lin_ps[:, bc, 0:out_l], x2[0:D, bs],
                                         wlin_sb[:], start=True, stop=True)

                    ybase = ypool.tile([128, NBH, out_l], F16, tag="yb")
                    nc.vector.tensor_add(ybase[:], lin_ps[:, :, 0:out_l],
                                         bfull_sb[li][:])

                    y_sb = ypool.tile([128, NBH, out_l], F32 if last else F16,
                                      tag=f"y{'2' if last else ''}")

                    if not last:
                        # combine: P = H * xmac (broadcast over o, il innermost),
                        # reduce over il, add lin+bias
                        hsb = hpool.tile([128, NBH, M], F16, tag="hs")
                        for bc in range(NBH):
                            nc.scalar.copy(hsb[:, bc, :], hps[bc][:])
                        psc = hpool.tile([128, NBH, out_l, I_PER], F16, tag="p")
                        nc.vector.tensor_mul(
                            psc[:],
                            hsb[:].rearrange("p bc (o il) -> p bc o il", il=I_PER),
                            xmac[:].unsqueeze(2).broadcast_to(
                                (128, NBH, out_l, I_PER)),
                        )
                        red_t = ypool.tile([128, NBH, out_l], F32, tag="red")
                        nc.vector.tensor_reduce(
                            red_t[:], psc[:], axis=mybir.AxisListType.X,
                            op=ADD)
                        nc.vector.tensor_add(y_sb[:], red_t[:], ybase[:])

                        # AllReduce (512, 64); plain batch-major return DMA
                        bounce = dpool.tile([HB, D], F16, tag=f"bounce{h}")
                        red = dpool.tile([HB, D], F16, tag=f"red{h}")
                        nc.sync.dma_start(
                            bounce[:].rearrange("(bc p) o -> p bc o", p=128),
                            y_sb[:],
                        )
                        nc.gpsimd.collective_compute(
                            "AllReduce",
                            ADD,
                            replica_groups=[list(range(NCORES))],
                            ins=[bounce.opt()],
                            outs=[red.opt()],
                        )
                        xs_n = xpool.tile([128, NBH, D], F16, tag=f"xsh{h}")
                        nc.sync.dma_start(
                            xs_n[:],
                            red[:].rearrange("(bc p) f -> p bc f", p=128),
                        )
                        xs_t[h] = xs_n
                    else:
                        h2_sb = hpool.tile([M, HB], F16, tag="hs")
                        nc.scalar.copy(h2_sb[:], h2_ps[:])
                        for bc in range(NBH):
                            bs = slice(bc * 128, (bc + 1) * 128)
                            t_ps = ps_rep.tile([128, M], F16, tag="rep")
                            nc.tensor.transpose(t_ps[:], h2_sb[:, bs],
                                                ident16_sb[0:M, 0:M])
                            h2c = qpool.tile([128, M], F16, tag="rk")
                            nc.scalar.copy(h2c[:], t_ps[:])
                            p2 = ypool.tile([128, out_l, I_PER], F16, tag="p2")
                            nc.vector.tensor_mul(
                                p2[:],
                                h2c[:].rearrange("p (o il) -> p o il", il=I_PER),
                                xmac[:, bc, :].unsqueeze(1).broadcast_to(
                                    (128, out_l, I_PER)),
                            )
                            r2 = ypool.tile([128, out_l], F32, tag="r2")
                            nc.vector.tensor_reduce(
                                r2[:], p2[:], axis=mybir.AxisListType.X, op=ADD)
                            nc.vector.tensor_add(y_sb[:, bc, :], r2[:],
                                                 ybase[:, bc, :])
                        nc.sync.dma_start(
                            out_ext.ap()[h * HB:(h + 1) * HB, :]
                            .rearrange("(bc p) o -> p bc o", p=128),
                            y_sb[:],
                        )

    nc.compile()
    return nc


# ------------------------------------------------------------------- runner --

def build_in_maps(x, W0, b0, W1, b1, W2, b2):
    x = np.asarray(x, np.float32)
    Ws = [np.asarray(W, np.float32) for W in (W0, W1, W2)]
    bs = [np.asarray(b_, np.float32) for b_ in (b0, b1, b2)]

    x16 = x.astype(F16NP)
    xdup = np.concatenate([x16, x16], axis=1)          # (B, 128)

    wcubs, wlins = {}, {}
    for li in range(3):
        wcubs[li], wlins[li] = _prep_layer(Ws[li], bs[li], OUTS[li])

    in_maps = []
    for core in range(NCORES):
        I = np.arange(core * I_PER, (core + 1) * I_PER)
        colsel = np.zeros((D, I_PER), np.float32)
        colsel[I, np.arange(I_PER)] = 1.0
        xmac0 = np.ascontiguousarray(
            x16[:, I].reshape(B // 128, 128, I_PER).transpose(1, 0, 2))
        m = {"xdup": xdup, "colsel": colsel.astype(F16NP), "xmac0": xmac0}
        for li in range(3):
            m[f"wcub{li}"] = wcubs[li][core]
            m[f"wlin{li}"] = wlins[li][core]
            bf = np.zeros((128, NBH, OUTS[li]), np.float32)
            if core == 0:
                bf[:] = bs[li][None, None, :]
            m[f"bfull{li}"] = bf.astype(F16NP)
        in_maps.append(m)
    return in_maps


def kernel(x, W0, b0, W1, b1, W2, b2):
    from concourse.bass_utils import run_bass_kernel_spmd

    if "nc" not in _CACHE:
        _CACHE["nc"] = _build_module()
    nc = _CACHE["nc"]

    in_maps = build_in_maps(x, W0, b0, W1, b1, W2, b2)
    res = run_bass_kernel_spmd(nc, in_maps, core_ids=list(range(NCORES)))
    out = np.zeros((B, OUTS[2]), np.float32)
    for core in range(NCORES):
        out += res.results[core]["out"]
    return out
